# revision 9
# baseline (speedup 1.0000x reference)
"""CoAttention ImageDNS kernel for Trainium2 (8 NeuronCores, Bass/Tile).

Math: the reference computes two additive-attention blocks. In both, the
softmax'd score is  score[b, q, k] = f(q-side)[b, q] + g(k-side)[b, k] + c,
and softmax over k is invariant to the q-dependent (and constant) terms, so
the attention weights are independent of the query index:

  visual_att[b, s, :]  = softmax_r( wB . tanh(W_i1 @ img[b, r]) )
  textual_att[b, i, :] = softmax_j( wD . tanh(W_d2 @ dns[b, j]) )

Hence both outputs are per-batch rank-1 broadcasts:

  att_img_features[b, s, :] = visual_att[b]  @ img[b]   (same for all s)
  att_dns_features[b, i, :] = textual_att[b] @ dns[b]   (same for all i)

W_d1/b_d1/w_att1[:H]/b_att1/W_i2/b_i2/w_att2[:H]/b_att2 cancel entirely.

Sharding: pure data-parallel over batch, 4 batches per core, no collectives.

Device dataflow (per core), designed around the bf16 PE streaming roofline
(~216 ns per K=128 N=512 matmul; LDWEIGHTS hides under the stream):
  - Only the h-transposed activations xt[h, row] are loaded (bf16); the rows
    of all 4 batches are packed along the free dim so row-chunks of 128 have
    no per-batch padding waste (784 img rows -> 7 chunks, 2048 dns -> 16).
  - Projection: chunk-major MMs, activations stationary, weights streaming.
  - score chain per chunk: tanh (ScalarE, bf16 out) -> scalar_tensor_tensor
    with the wB/wD broadcast row + free-dim accumulate (VectorE) giving the
    score column [rk, 1]; a PE transpose turns it into a score row; exp on
    ScalarE writes the per-side exp row [1, rows].
  - per batch: row-sum + reciprocal + normalize (VectorE), partition-
    broadcast of the normalized attention row (GpSimd), then stage-2 as 8
    STT free-dim-accumulate ops over xt (VectorE) - no xn loads, no PE.
  - outputs: one [H] vector per (batch, side), PE-transposed to [8, 128]
    and DMA'd out (32 KB total instead of 16.8 MB of broadcast rows); the
    host broadcasts to the full (B, S, H) shape during unshard.
  - PE-queue ops that depend on the VectorE chain (the transposes) are
    emitted 1-2 chunks late so the in-order PE queue never stalls.
"""

import sys
import numpy as np
import ml_dtypes

_BF16 = ml_dtypes.bfloat16

for _p in ("/opt/trn_rl_repo", "/root/.axon_site/_ro/trn_rl_repo"):
    if _p not in sys.path:
        sys.path.append(_p)

B, S, R, H = 32, 512, 196, 1024
NCORES = 8
BLOC = B // NCORES          # batches per core
OC = 512                    # output-chunk (one fp32 PSUM bank)
HC = H // 128               # contraction chunks
DTOT = BLOC * S             # packed dns rows per core (2048)
ITOT = BLOC * R             # packed img rows per core (784)

_CACHE = {}


def _row_chunks(n):
    out, o = [], 0
    while o < n:
        out.append((o, min(128, n - o)))
        o += 128
    return out


def build_nc():
    from concourse import bacc, mybir
    from concourse import tile

    f32, f16 = mybir.dt.float32, mybir.dt.bfloat16
    Act = mybir.ActivationFunctionType
    Alu = mybir.AluOpType
    Ax = mybir.AxisListType

    nc = bacc.Bacc("TRN2", target_bir_lowering=False, debug=False)

    xt_dns_d = nc.dram_tensor("xt_dns", [HC, 128, DTOT], f16, kind="ExternalInput")
    xt_img_d = nc.dram_tensor("xt_img", [HC, 128, ITOT], f16, kind="ExternalInput")
    wt_i1_d = nc.dram_tensor("wt_i1", [HC, 128, H], f16, kind="ExternalInput")
    wt_d2_d = nc.dram_tensor("wt_d2", [HC, 128, H], f16, kind="ExternalInput")
    wr_b_d = nc.dram_tensor("wrow_b", [1, H], f16, kind="ExternalInput")
    wr_d_d = nc.dram_tensor("wrow_d", [1, H], f16, kind="ExternalInput")
    ident_d = nc.dram_tensor("ident", [128, 128], f32, kind="ExternalInput")
    out_d = nc.dram_tensor("out_all", [2, BLOC, HC, 128], f32, kind="ExternalOutput")

    IMG_RCS = _row_chunks(ITOT)      # 7 chunks (6x128 + 16)
    DNS_RCS = _row_chunks(DTOT)      # 16 chunks
    # batch -> last chunk holding its rows (chunk boundaries don't align with
    # batch boundaries on the img side)
    img_last_chunk = [((b + 1) * R - 1) // 128 for b in range(BLOC)]
    dns_last_chunk = [((b + 1) * S - 1) // 128 for b in range(BLOC)]

    with tile.TileContext(nc) as tc:
        with (
            tc.tile_pool(name="const", bufs=1) as cpool,
            tc.tile_pool(name="th", bufs=3) as thpool,
            tc.tile_pool(name="scr", bufs=2) as jpool,
            tc.tile_pool(name="small", bufs=3) as spool,
            tc.tile_pool(name="attp", bufs=4) as apool,
            tc.tile_pool(name="attacc", bufs=8) as accpool,
            tc.tile_pool(name="bc", bufs=2) as bpool,
            tc.tile_pool(name="pp", bufs=3, space="PSUM") as ppool,
            tc.tile_pool(name="tp", bufs=2, space="PSUM") as tpool,
        ):
            # ---- persistent SBUF tiles ----
            xt_img = cpool.tile([128, HC, ITOT], f16, name="xt_img_sb")
            xt_dns = cpool.tile([128, HC, DTOT], f16, name="xt_dns_sb")
            wt_img = cpool.tile([128, HC, H], f16, name="wt_img_sb")
            wt_dns = cpool.tile([128, HC, H], f16, name="wt_dns_sb")
            wr_row = {"img": cpool.tile([1, H], f16, name="wr_row_img"),
                      "dns": cpool.tile([1, H], f16, name="wr_row_dns")}
            wrb = {"img": cpool.tile([128, H], f16, name="wrb_img"),
                   "dns": cpool.tile([128, H], f16, name="wrb_dns")}
            ident = cpool.tile([128, 128], f32, name="ident_sb")
            erow = {"img": cpool.tile([1, ITOT], f32, name="erow_img"),
                    "dns": cpool.tile([1, DTOT], f32, name="erow_dns")}

            SD = {
                "img": dict(xt=xt_img, wt=wt_img, rcs=IMG_RCS, rows=R,
                            last=img_last_chunk, oidx=0),
                "dns": dict(xt=xt_dns, wt=wt_dns, rcs=DNS_RCS, rows=S,
                            last=dns_last_chunk, oidx=1),
            }

            # ---- PE warmup: ~10 dummy matmuls on scratch data get the HAM
            # clock gate to 8/8 (2.4 GHz) during the first ~4us, which is
            # DMA-bound anyway; real matmuls then run warm from the start.
            warm_sb = cpool.tile([128, OC], f16, name="warm_sb")
            nc.vector.memset(warm_sb[:, :], 0.0)
            warm_ps = ppool.tile([128, H], f32, name="warm_ps", tag="pp")
            for i in range(10):
                nc.tensor.matmul(warm_ps[:, 0:OC], lhsT=warm_sb[:, 0:128],
                                 rhs=warm_sb[:, :], start=True, stop=True)

            # ---- input DMAs ----
            # scalar queue: small constants needed by the early score chain
            nc.scalar.dma_start(out=ident[:, :], in_=ident_d[:, :])
            nc.scalar.dma_start(out=wr_row["img"][:, :], in_=wr_b_d[:, :])
            nc.scalar.dma_start(out=wr_row["dns"][:, :], in_=wr_d_d[:, :])
            # img side per-hc on sync (xt) + gpsimd (wt) so the hc-major
            # prologue can start as soon as the first slices land
            for hc in range(HC):
                nc.gpsimd.dma_start(out=wt_img[:, hc, :], in_=wt_i1_d[hc])
                nc.sync.dma_start(out=xt_img[:, hc, :], in_=xt_img_d[hc])
            # wB/wD broadcast rows (gpsimd queue, after the img wt issues)
            for s in ("img", "dns"):
                nc.gpsimd.partition_broadcast(wrb[s][:, :], wr_row[s][0:1, :])
            # dns side: wt on gpsimd; xt on sync in two column halves so the
            # first dns chunks' data lands before img compute drains
            for hc in range(HC):
                nc.gpsimd.dma_start(out=wt_dns[:, hc, :], in_=wt_d2_d[hc])
            HD = DTOT // 2
            for half in range(2):
                cs = slice(half * HD, (half + 1) * HD)
                for hc in range(HC):
                    nc.sync.dma_start(out=xt_dns[:, hc, cs],
                                      in_=xt_dns_d[hc][:, cs])

            # ---- per-chunk pieces ----
            tcols = {}
            tps_tiles = {}

            def emit_mm(side, ci):
                """proj MMs for one chunk + its (non-PE) score chain."""
                sd = SD[side]
                r0, rk = sd["rcs"][ci]
                ps = ppool.tile([128, H], f32, name=f"ps_{side}_{ci}", tag="pp")
                for hc in range(HC):
                    lhs = sd["xt"][:, hc, r0:r0 + rk]
                    for oc in range(2):
                        nc.tensor.matmul(
                            ps[0:rk, oc * OC:(oc + 1) * OC],
                            lhsT=lhs,
                            rhs=sd["wt"][:, hc, oc * OC:(oc + 1) * OC],
                            start=(hc == 0), stop=(hc == HC - 1))
                emit_chain(side, ci, ps)

            def emit_mm_prologue(side, cis):
                """hc-major MMs over several chunks: consumes the per-hc input
                DMAs progressively so the PE starts ~1.5us into the kernel."""
                sd = SD[side]
                pss = {}
                for ci in cis:
                    pss[ci] = ppool.tile([128, H], f32, name=f"ps_{side}_{ci}",
                                         tag="pp")
                for hc in range(HC):
                    for ci in cis:
                        r0, rk = sd["rcs"][ci]
                        lhs = sd["xt"][:, hc, r0:r0 + rk]
                        for oc in range(2):
                            nc.tensor.matmul(
                                pss[ci][0:rk, oc * OC:(oc + 1) * OC],
                                lhsT=lhs,
                                rhs=sd["wt"][:, hc, oc * OC:(oc + 1) * OC],
                                start=(hc == 0), stop=(hc == HC - 1))
                for ci in cis:
                    emit_chain(side, ci, pss[ci])

            def emit_chain(side, ci, ps):
                """tanh -> weighted free-dim reduce -> score column [rk, 1]."""
                sd = SD[side]
                r0, rk = sd["rcs"][ci]
                th = thpool.tile([128, H], f16, name=f"th_{side}_{ci}", tag="th")
                nc.scalar.activation(th[0:rk, :], ps[0:rk, :], Act.Tanh)
                scr = jpool.tile([128, H], f16, name=f"scr_{side}_{ci}", tag="scr")
                tcol = spool.tile([128, 1], f32, name=f"tc_{side}_{ci}", tag="tcol")
                nc.vector.scalar_tensor_tensor(
                    out=scr[0:rk, :], in0=th[0:rk, :], scalar=1.0,
                    in1=wrb[side][0:rk, :], op0=Alu.mult, op1=Alu.mult,
                    accum_out=tcol[0:rk, :])
                tcols[(side, ci)] = tcol

            def emit_T(side, ci):
                """PE transpose of the score column -> exp row slice.
                Emitted >=1 chunk after emit_mm so the PE queue never waits
                on the VectorE chain."""
                sd = SD[side]
                r0, rk = sd["rcs"][ci]
                tcol = tcols[(side, ci)]
                tps = tpool.tile([8, 128], f32, name=f"tps_{side}_{ci}", tag="tp")
                nc.tensor.transpose(tps[0:1, 0:rk], tcol[0:rk, 0:1],
                                    ident[0:rk, 0:rk])
                nc.scalar.activation(erow[side][0:1, r0:r0 + rk],
                                     tps[0:1, 0:rk], Act.Exp)

            # ---- stage 2, split into head/tail parts ----
            # att[h] = (sum_r exp_r x[h,r]) / sum_r exp_r.  The unnormalized
            # partials only need the exp row, so the head part (all chunks of
            # the batch but the last) runs a chunk earlier than a normalized
            # formulation would allow; only the last chunk's sliver plus the
            # finalize remains on the critical tail.
            attps, atts = {}, {}

            def emit_part(side, b, head):
                sd = SD[side]
                rows = sd["rows"]
                b0, bend = b * rows, (b + 1) * rows
                split = max(b0, sd["last"][b] * 128)
                lo, hi = (b0, split) if head else (split, bend)
                if hi <= lo:
                    return
                w = hi - lo
                key = (side, b)
                if key not in attps:
                    attp = accpool.tile([128, HC * 2], f32,
                                        name=f"attp_{side}_{b}", tag="attp")
                    nc.vector.memset(attp[:, :], 0.0)
                    attps[key] = attp
                attp = attps[key]
                pi = 0 if head else 1
                abc = bpool.tile([128, w], f32, name=f"abc_{side}_{b}_{pi}",
                                 tag=f"abc_{int(head)}_{side}")
                nc.gpsimd.partition_broadcast(abc[:, :], erow[side][0:1, lo:hi])
                for hc in range(HC):
                    sj = jpool.tile([128, w], f16, name=f"sj_{side}_{b}_{hc}_{pi}",
                                    tag=f"sj_{side}")
                    nc.vector.scalar_tensor_tensor(
                        out=sj[:, :], in0=sd["xt"][:, hc, lo:hi],
                        scalar=1.0, in1=abc[:, :], op0=Alu.mult, op1=Alu.mult,
                        accum_out=attp[:, hc * 2 + pi:hc * 2 + pi + 1])

            def emit_finalize(side, b):
                sd = SD[side]
                rows = sd["rows"]
                b0 = b * rows
                ssum = spool.tile([1, 1], f32, name=f"ss_{side}_{b}", tag="ssum")
                nc.vector.tensor_reduce(out=ssum[0:1, 0:1],
                                        in_=erow[side][0:1, b0:b0 + rows],
                                        axis=Ax.X, op=Alu.add)
                rcp = spool.tile([1, 1], f32, name=f"rc_{side}_{b}", tag="rcp")
                nc.vector.reciprocal(rcp[0:1, 0:1], ssum[0:1, 0:1])
                rb = spool.tile([128, 1], f32, name=f"rb_{side}_{b}", tag="rb")
                nc.gpsimd.partition_broadcast(rb[:, 0:1], rcp[0:1, 0:1])
                attp = attps[(side, b)]
                attf = apool.tile([128, HC], f32, name=f"attf_{side}_{b}",
                                  tag="attf")
                nc.vector.tensor_reduce(
                    out=attf[:, :],
                    in_=attp[:, :].rearrange("p (h t) -> p h t", t=2),
                    axis=Ax.X, op=Alu.add)
                attsc = apool.tile([128, HC], f32, name=f"attsc_{side}_{b}",
                                   tag="attsc")
                nc.scalar.activation(attsc[:, :], attf[:, :], Act.Copy,
                                     scale=rb[:, 0:1])
                atts[(side, b)] = attsc

            def emit_attT(side, b):
                """PE transpose of the output column-tile + writeback."""
                att = atts[(side, b)]
                atp = tpool.tile([8, 128], f32, name=f"atp_{side}_{b}", tag="tp")
                nc.tensor.transpose(atp[0:8, 0:128], att[:, 0:HC],
                                    ident[:, :])
                osb = spool.tile([8, 128], f32, name=f"osb_{side}_{b}", tag="osb")
                nc.scalar.activation(osb[:, :], atp[:, :], Act.Copy)
                nc.sync.dma_start(out=out_d[SD[side]["oidx"], b], in_=osb[:, :])

            def after_T(side, ci):
                for b in range(BLOC):
                    if SD[side]["last"][b] - 1 == ci:
                        emit_part(side, b, head=True)
                    if SD[side]["last"][b] == ci:
                        emit_part(side, b, head=False)
                        emit_finalize(side, b)

            # ---- emission schedule ----
            # PE order: img prologue (c0-2, hc-major, DMA-paced), img c3-c5,
            # all dns chunks (their xt arrives while img computes), and the
            # 16-row img c6 last so the end-of-kernel chain is short.  Score
            # transposes are deferred >=1 chunk; output transposes >=2.
            emit_mm_prologue("img", [0, 1, 2])
            emit_mm("img", 3)
            for ci in (0, 1, 2):
                emit_T("img", ci); after_T("img", ci)
            emit_mm("img", 4); emit_T("img", 3); after_T("img", 3)
            emit_mm("img", 5); emit_T("img", 4); after_T("img", 4)
            emit_mm("dns", 0); emit_T("img", 5); after_T("img", 5)
            emit_mm("dns", 1); emit_attT("img", 0)
            emit_mm("dns", 2); emit_attT("img", 1); emit_T("dns", 0); after_T("dns", 0)
            emit_mm("dns", 3); emit_attT("img", 2); emit_T("dns", 1); after_T("dns", 1)
            attT_slot = {7: ("dns", 0), 11: ("dns", 1), 15: ("dns", 2)}
            for ci in range(4, 16):
                emit_mm("dns", ci)
                if ci in attT_slot:
                    emit_attT(*attT_slot[ci])
                emit_T("dns", ci - 2); after_T("dns", ci - 2)
            emit_mm("img", 6)
            emit_T("dns", 14); after_T("dns", 14)
            emit_T("dns", 15); after_T("dns", 15)
            emit_T("img", 6); after_T("img", 6)
            emit_attT("dns", 3)
            emit_attT("img", 3)

    nc.compile()
    return nc


def _get_nc():
    if "nc" not in _CACHE:
        _CACHE["nc"] = build_nc()
    return _CACHE["nc"]


def make_in_maps(inputs):
    dns = np.asarray(inputs["dns_feature"], dtype=np.float32)
    img = np.asarray(inputs["img_features"], dtype=np.float32)
    W_i1 = np.asarray(inputs["W_i1"], dtype=np.float32)
    W_d2 = np.asarray(inputs["W_d2"], dtype=np.float32)
    wB = np.asarray(inputs["w_att1"], dtype=np.float32)[H:]
    wD = np.asarray(inputs["w_att2"], dtype=np.float32)[H:]

    wt_i1 = np.ascontiguousarray(W_i1.T).reshape(HC, 128, H).astype(_BF16)
    wt_d2 = np.ascontiguousarray(W_d2.T).reshape(HC, 128, H).astype(_BF16)
    wr_b = np.ascontiguousarray(wB.reshape(1, H).astype(_BF16))
    wr_d = np.ascontiguousarray(wD.reshape(1, H).astype(_BF16))
    ident = np.eye(128, dtype=np.float32)

    in_maps = []
    for k in range(NCORES):
        sl = slice(k * BLOC, (k + 1) * BLOC)
        xd = np.ascontiguousarray(
            dns[sl].transpose(2, 0, 1).reshape(HC, 128, DTOT).astype(_BF16))
        xi = np.ascontiguousarray(
            img[sl].transpose(2, 0, 1).reshape(HC, 128, ITOT).astype(_BF16))
        in_maps.append({
            "xt_dns": xd, "xt_img": xi,
            "wt_i1": wt_i1, "wt_d2": wt_d2,
            "wrow_b": wr_b, "wrow_d": wr_d, "ident": ident,
        })
    return in_maps


def kernel(**inputs):
    from concourse.bass_utils import run_bass_kernel_spmd

    nc = _get_nc()
    in_maps = make_in_maps(inputs)
    res = run_bass_kernel_spmd(nc, in_maps, list(range(NCORES))).results
    out = np.stack([np.asarray(res[k]["out_all"]) for k in range(NCORES)])
    img_rows = out[:, 0].reshape(B, H)
    dns_rows = out[:, 1].reshape(B, H)
    att_dns = np.ascontiguousarray(
        np.broadcast_to(dns_rows[:, None, :], (B, S, H)))
    att_img = np.ascontiguousarray(
        np.broadcast_to(img_rows[:, None, :], (B, S, H)))
    return att_dns, att_img
